# revision 3
# baseline (speedup 1.0000x reference)
"""Locally-connected Conv2d (unique weights per output location) on 8 trn2 cores.

Problem (hardcoded): x [256,1,280,280] f32, weight [12800,1,28,28] f32,
bias [12800,1] f32 -> out [256,128,10,10] f32.  kernel 28x28, stride 28
(non-overlapping patches), 10x10=100 locations, 128 filters.

Per location l the computation is a plain matmul:
    out[b, f, l] = sum_k patch[b, l, k] * w[f, l, k] + bias[f, l],  k in [0,784)

Strategy: shard the 100 locations across 8 cores (pad to 104 = 8*13).
Host-side we repack x into k-major patch layout and weights into k-major
filter layout (both fp16 to halve HBM traffic; accumulation is fp32 in
PSUM).  The bias is folded into the contraction as a 113th k-row
(x row 112 = 1.0, w row 112 = bias on chunk 0 / zero on chunks 1-6), so
the device does nothing but 7 uniform accumulating matmuls per location:
    per location: 7 matmuls [113k x 128f]^T @ [113k x 256b] -> PSUM fp32

The kernel is DMA-bound (~8.7 MB/core at ~270 GB/s effective).  Loads
are split per location-pair (7 pairs of ~1.2 MB) and drained in issue
order on the SWDGE queue, so compute on pair p overlaps the loads of
pairs p+1..; the single-location pair goes last to shorten the tail.
Stores ride the two HWDGE queues (sync/scalar) so each store's only
sync-wait is its data dependency (this walrus build caps instructions
at ONE sync-wait; SW-lane reuse past the 8th DMA would add a second).

Environment notes (this walrus build / axon runtime):
  - each DMA / matmul / ldweights / Pool-copy instruction may carry at
    most ONE sync-wait command.
  - SWDGE (nc.gpsimd) sprays a dma_start across all 16 SDMA engines;
    queue packets drain in issue order.
"""

import numpy as np

import concourse.bass as bass
import concourse.mybir as mybir
from concourse import bass_utils
from concourse.tile import TileContext
from concourse.vector_clock import ScopedClock


def _split_drain_and_barrier(self, tick_clock, wait_clock):
    """TileContext._drain_and_barrier with the tail drain's sem waits split
    across several drain instructions: this walrus build caps the number of
    sync-wait commands a single instruction may carry."""
    drain_inst = self.nc.sync.drain()
    wait_clock.add_sem_waits(
        drain_inst.ins, ScopedClock({None: tick_clock.global_clock}))
    mi = drain_inst.ins
    if mi.sync_info is not None and mi.sync_info.on_wait:
        waits = list(mi.sync_info.on_wait)
        ups = list(mi.sync_info.on_update or [])
        mi.sync_info = mybir.SyncInfo(on_wait=waits[:1], on_update=ups)
        for w in waits[1:]:
            extra = self.nc.sync.drain()
            extra.ins.sync_info = mybir.SyncInfo(on_wait=[w], on_update=[])
    self.nc.all_engine_barrier(sem_only=True)
    assert self.sems is not None
    popped = self.nc._tile_sem_poison_stack.pop()
    assert popped is self._sem_poison
    if not SKIP_TAIL_CLEAR:
        self.nc.clear_and_free_semaphores(list(self.sems.allocated().values()))
        self.nc.all_engine_barrier(sem_only=True)


SKIP_TAIL_CLEAR = True

TileContext._drain_and_barrier = _split_drain_and_barrier

B = 256       # batch
NF = 128      # filters
HS = WS = 10  # output spatial
L = HS * WS   # locations
KH = KW = 28  # kernel == stride (non-overlapping)
NCORES = 8
LPC = 13      # locations per core (8*13 = 104 >= 100, tail zero-padded)
LPAD = NCORES * LPC
KC = 7        # contraction chunks
KP = 113      # partitions per chunk: 112 data rows (kh splits as (7,4)) + 1
              # bias row (x=1.0; w=bias on chunk 0, 0 elsewhere)

# location pairs per core: 6 pairs of 2 + 1 single (pairs share a PSUM bank;
# the single-location pair is LAST so the post-DMA compute tail is short)
NPL = [2, 2, 2, 2, 2, 2, 1]
NPAIR = len(NPL)

# pair 0 loads ride the HWDGE queues (sync/scalar): HWDGE has no Q7
# descriptor-generation spin-up, so its first bytes land ~4 us before the
# SWDGE stream starts, buying a head start on the DMA-bound critical path
HWDGE_HEADSTART = True

_CACHED = {}


def _build_bass():
    nc = bass.Bass(trn_type="TRN2")
    xs = [nc.dram_tensor(f"xp{p}", [KP, NPL[p], KC, B], mybir.dt.float16,
                         kind="ExternalInput")
          for p in range(NPAIR)]
    ws = [nc.dram_tensor(f"wp{p}", [KP, NPL[p], KC, NF], mybir.dt.float16,
                         kind="ExternalInput")
          for p in range(NPAIR)]
    # separate store tensors: avoids per-tensor WAW chaining between stores
    outs = [nc.dram_tensor(f"op{p}", [NF, NPL[p], B], mybir.dt.float16,
                           kind="ExternalOutput")
            for p in range(NPAIR)]

    with TileContext(nc) as tc:
        with (
            tc.tile_pool(name="xp", bufs=1) as xpool,
            tc.tile_pool(name="wp", bufs=1) as wpool,
            tc.tile_pool(name="op", bufs=1) as opool,
            # 2 locations share one PSUM bank: NPAIR=7 tiles <= 8 banks, so
            # banks are never reused and matmuls need no release wait.
            tc.tile_pool(name="ps", bufs=1, space="PSUM") as pspool,
        ):
            x_ts, w_ts = [], []
            for p in range(NPAIR):
                x_t = xpool.tile([KP, NPL[p], KC, B], mybir.dt.float16,
                                 tag=f"x{p}", name=f"x{p}")
                w_t = wpool.tile([KP, NPL[p], KC, NF], mybir.dt.float16,
                                 tag=f"w{p}", name=f"w{p}")
                if p == 0 and HWDGE_HEADSTART:
                    nc.sync.dma_start(x_t[:], xs[p][:])
                    nc.scalar.dma_start(w_t[:], ws[p][:])
                else:
                    nc.gpsimd.dma_start(x_t[:], xs[p][:])
                    nc.gpsimd.dma_start(w_t[:], ws[p][:])
                x_ts.append(x_t)
                w_ts.append(w_t)

            # HW-DGE lane budget is 8: 2 head-start loads + 6 stores.  The
            # last store goes SWDGE, its data wait laundered through a Pool
            # carrier op so the DMA carries only its lane-reuse wait.
            carrier = opool.tile([1, 1], mybir.dt.float16, tag="carrier",
                                 name="carrier")
            for p in range(NPAIR):
                npl = NPL[p]
                ps = pspool.tile([NF, npl, B], mybir.dt.float32,
                                 tag=f"ps{p}", name=f"ps{p}")
                for j in range(npl):
                    for c in range(KC):
                        nc.tensor.matmul(ps[:, j, :], w_ts[p][:, j, c, :],
                                         x_ts[p][:, j, c, :],
                                         start=(c == 0), stop=(c == KC - 1))
                o_t = opool.tile([NF, npl, B], mybir.dt.float16,
                                 tag=f"o{p}", name=f"o{p}")
                nc.vector.tensor_copy(o_t[:], ps[:])
                if p < NPAIR - 1:
                    eng = nc.sync if p % 2 == 0 else nc.scalar
                    eng.dma_start(outs[p][:], o_t[:])
                else:
                    nc.gpsimd.tensor_copy(carrier[:], o_t[0:1, 0, 0:1])
                    nc.gpsimd.dma_start(outs[p][:], o_t[:])
    return nc


def _pack_inputs(x, weight, bias):
    # x: [B,1,280,280] f32.  rows = i*28 + kh, kh = c*4 + khm; cols = j*28 + kw
    # xk[p, l=(i,j), c, b] fp16 with p = khm*28 + kw;  row 112 = 1.0 (bias)
    xh = x.astype(np.float16).reshape(B, HS, KC, 4, WS, KW)
    # (b, i, c, khm, j, kw) -> (khm, kw, i, j, c, b)
    xt = np.ascontiguousarray(xh.transpose(3, 5, 1, 4, 2, 0))
    xkf = np.zeros((KP, LPAD, KC, B), np.float16)
    xkf[:112, :L] = xt.reshape(112, L, KC, B)
    xkf[112] = 1.0

    # weight: [NF*L, 1, 28, 28] -> [f, l, c, khm, kw] -> [(khm,kw), l, c, f]
    # row 112 = bias (chunk 0) / 0 (chunks 1-6)
    wh = weight.astype(np.float16).reshape(NF, L, KC, 4, KW)
    wt = np.ascontiguousarray(wh.transpose(3, 4, 1, 2, 0)).reshape(112, L, KC, NF)
    wkf = np.zeros((KP, LPAD, KC, NF), np.float16)
    wkf[:112, :L] = wt
    wkf[112, :L, 0, :] = bias.astype(np.float16).reshape(NF, L).T

    in_maps = []
    for c in range(NCORES):
        base = c * LPC
        m = {}
        off = 0
        for p in range(NPAIR):
            s0, s1 = base + off, base + off + NPL[p]
            m[f"xp{p}"] = np.ascontiguousarray(xkf[:, s0:s1])
            m[f"wp{p}"] = np.ascontiguousarray(wkf[:, s0:s1])
            off += NPL[p]
        in_maps.append(m)
    return in_maps


def run(x, weight, bias, **run_kwargs):
    """Build+run; returns (output, BassKernelResults)."""
    if "nc" not in _CACHED:
        _CACHED["nc"] = _build_bass()
    nc = _CACHED["nc"]
    in_maps = _pack_inputs(x, weight, bias)
    res = bass_utils.run_bass_kernel_spmd(
        nc, in_maps, core_ids=list(range(NCORES)), **run_kwargs)
    # per core: op{p} is [NF, npl, B]; concat -> [NF, LPC, B]
    outs = np.stack([
        np.concatenate([r[f"op{p}"] for p in range(NPAIR)], axis=1)
        for r in res.results])                        # [8, NF, LPC, B]
    outs = outs.transpose(0, 2, 1, 3).reshape(LPAD, NF, B)[:L]  # [l, f, b]
    out = np.ascontiguousarray(outs.transpose(2, 1, 0)).reshape(B, NF, HS, WS)
    return out.astype(np.float32), res


def kernel(x, weight, bias):
    out, _ = run(x, weight, bias)
    return out


# revision 4
# speedup vs baseline: 2.9537x; 2.9537x over previous
"""Locally-connected Conv2d (unique weights per output location) on 8 trn2 cores.

Problem (hardcoded): x [256,1,280,280] f32, weight [12800,1,28,28] f32,
bias [12800,1] f32 -> out [256,128,10,10] f32.  kernel 28x28, stride 28
(non-overlapping patches), 10x10=100 locations, 128 filters.

Per location l the computation is a plain matmul:
    out[b, f, l] = sum_k patch[b, l, k] * w[f, l, k] + bias[f, l],  k in [0,784)

Strategy: shard the 100 locations across 8 cores (pad to 104 = 8*13).
Host-side we repack x into k-major patch layout and weights into k-major
filter layout (both fp16 to halve HBM traffic; accumulation is fp32 in
PSUM), so the device does nothing but streaming matmuls:
    per location: 7 accumulating matmuls [112k x 128f]^T @ [112k x 256b]
    + one K=1 matmul (ones x bias row) that folds in the bias.

The kernel is DMA-bound (~8.7 MB/core at ~270 GB/s effective).  Loads are
split per location-pair (7 pairs of ~1.2 MB) and drained in issue order
on the SWDGE queue, so compute on pair p overlaps the loads of pairs
p+1..; the single-location pair goes last to shorten the post-DMA tail.
Stores ride the HWDGE queues so each store's only sync-wait is its data
dependency; the last store goes SWDGE with its wait laundered through a
Pool carrier op.

Environment notes (this walrus build / axon runtime):
  - each DMA / matmul / ldweights / Pool-copy instruction may carry at
    most ONE sync-wait command; HW-DGE/SW-DGE lane groups have 8 lanes
    each and DMAs past the 8th on a group get a lane-reuse wait.
  - DMA partition counts MUST split evenly over the 16 SDMA engines
    (112 or 128): a 113-partition transfer shreds the leftover row into
    4-8 byte descriptors and collapses DMA bandwidth ~5x.
  - SWDGE (nc.gpsimd) sprays a dma_start across all 16 SDMA engines and
    queue packets drain in issue order; HWDGE handles 128-partition
    tiles fine but is not used for bulk loads here.
"""

import numpy as np

import concourse.bass as bass
import concourse.mybir as mybir
from concourse import bass_utils
from concourse.tile import TileContext
from concourse.vector_clock import ScopedClock


def _split_drain_and_barrier(self, tick_clock, wait_clock):
    """TileContext._drain_and_barrier with the tail drain's sem waits split
    across several drain instructions: this walrus build caps the number of
    sync-wait commands a single instruction may carry."""
    drain_inst = self.nc.sync.drain()
    wait_clock.add_sem_waits(
        drain_inst.ins, ScopedClock({None: tick_clock.global_clock}))
    mi = drain_inst.ins
    if mi.sync_info is not None and mi.sync_info.on_wait:
        waits = list(mi.sync_info.on_wait)
        ups = list(mi.sync_info.on_update or [])
        mi.sync_info = mybir.SyncInfo(on_wait=waits[:1], on_update=ups)
        for w in waits[1:]:
            extra = self.nc.sync.drain()
            extra.ins.sync_info = mybir.SyncInfo(on_wait=[w], on_update=[])
    self.nc.all_engine_barrier(sem_only=True)
    assert self.sems is not None
    popped = self.nc._tile_sem_poison_stack.pop()
    assert popped is self._sem_poison
    if not SKIP_TAIL_CLEAR:
        self.nc.clear_and_free_semaphores(list(self.sems.allocated().values()))
        self.nc.all_engine_barrier(sem_only=True)


SKIP_TAIL_CLEAR = True

TileContext._drain_and_barrier = _split_drain_and_barrier

B = 256       # batch
NF = 128      # filters
HS = WS = 10  # output spatial
L = HS * WS   # locations
KH = KW = 28  # kernel == stride (non-overlapping)
NCORES = 8
LPC = 13      # locations per core (8*13 = 104 >= 100, tail zero-padded)
LPAD = NCORES * LPC
KC = 7        # contraction chunks
KP = 112      # partitions per chunk (7*112 = 784); kh splits as (7,4)

# location pairs per core: 6 pairs of 2 + 1 single (pairs share a PSUM bank;
# the single-location pair is LAST so the post-DMA compute tail is short)
NPL = [2, 2, 2, 2, 2, 2, 1]
NPAIR = len(NPL)

_CACHED = {}


def _build_bass():
    nc = bass.Bass(trn_type="TRN2")
    xs = [nc.dram_tensor(f"xp{p}", [KP, NPL[p], KC, B], mybir.dt.float16,
                         kind="ExternalInput")
          for p in range(NPAIR)]
    ws = [nc.dram_tensor(f"wp{p}", [KP, NPL[p], KC, NF], mybir.dt.float16,
                         kind="ExternalInput")
          for p in range(NPAIR)]
    bk = nc.dram_tensor("bk", [1, LPC, NF], mybir.dt.float16,
                        kind="ExternalInput")
    # separate store tensors: avoids per-tensor WAW chaining between stores
    outs = [nc.dram_tensor(f"op{p}", [NF, NPL[p], B], mybir.dt.float16,
                           kind="ExternalOutput")
            for p in range(NPAIR)]

    with TileContext(nc) as tc:
        with (
            tc.tile_pool(name="xp", bufs=1) as xpool,
            tc.tile_pool(name="wp", bufs=1) as wpool,
            tc.tile_pool(name="bp", bufs=1) as bpool,
            tc.tile_pool(name="op", bufs=1) as opool,
            # 2 locations share one PSUM bank: NPAIR=7 tiles <= 8 banks, so
            # banks are never reused and matmuls need no release wait.
            tc.tile_pool(name="ps", bufs=1, space="PSUM") as pspool,
        ):
            ones_t = bpool.tile([1, B], mybir.dt.float16, tag="ones",
                                name="ones")
            nc.vector.memset(ones_t[:], 1.0)
            bias_t = bpool.tile([1, LPC, NF], mybir.dt.float16, tag="bias",
                                name="bias")
            # tiny; rides HWDGE so the SWDGE queue stays pure bulk loads
            nc.sync.dma_start(bias_t[:], bk[:])

            x_ts, w_ts = [], []
            for p in range(NPAIR):
                x_t = xpool.tile([KP, NPL[p], KC, B], mybir.dt.float16,
                                 tag=f"x{p}", name=f"x{p}")
                w_t = wpool.tile([KP, NPL[p], KC, NF], mybir.dt.float16,
                                 tag=f"w{p}", name=f"w{p}")
                nc.gpsimd.dma_start(x_t[:], xs[p][:])
                nc.gpsimd.dma_start(w_t[:], ws[p][:])
                x_ts.append(x_t)
                w_ts.append(w_t)

            # HW-DGE lane budget is 8: 1 bias load + 6 stores.  The last
            # store goes SWDGE, its data wait laundered through a Pool
            # carrier op so the DMA carries only its lane-reuse wait.
            carrier = opool.tile([1, 1], mybir.dt.float16, tag="carrier",
                                 name="carrier")
            loc = 0
            for p in range(NPAIR):
                npl = NPL[p]
                ps = pspool.tile([NF, npl, B], mybir.dt.float32,
                                 tag=f"ps{p}", name=f"ps{p}")
                for j in range(npl):
                    for c in range(KC):
                        nc.tensor.matmul(ps[:, j, :], w_ts[p][:, j, c, :],
                                         x_ts[p][:, j, c, :],
                                         start=(c == 0), stop=False)
                    # bias: rank-1 update  ps[f, b] += bias[f] * 1
                    nc.tensor.matmul(ps[:, j, :], bias_t[:, loc + j, :],
                                     ones_t[:], start=False, stop=True)
                o_t = opool.tile([NF, npl, B], mybir.dt.float16,
                                 tag=f"o{p}", name=f"o{p}")
                nc.vector.tensor_copy(o_t[:], ps[:])
                if p < NPAIR - 1:
                    eng = nc.sync if p % 2 == 0 else nc.scalar
                    eng.dma_start(outs[p][:], o_t[:])
                else:
                    nc.gpsimd.tensor_copy(carrier[:], o_t[0:1, 0, 0:1])
                    nc.gpsimd.dma_start(outs[p][:], o_t[:])
                loc += npl
    return nc


def _pack_inputs(x, weight, bias):
    # x: [B,1,280,280] f32.  rows = i*28 + kh, kh = c*4 + khm; cols = j*28 + kw
    # xk[p, l=(i,j), c, b] fp16 with p = khm*28 + kw
    xh = x.astype(np.float16).reshape(B, HS, KC, 4, WS, KW)
    # (b, i, c, khm, j, kw) -> (khm, kw, i, j, c, b)
    xt = np.ascontiguousarray(xh.transpose(3, 5, 1, 4, 2, 0))
    xkf = np.zeros((KP, LPAD, KC, B), np.float16)
    xkf[:, :L] = xt.reshape(KP, L, KC, B)

    # weight: [NF*L, 1, 28, 28] -> [f, l, c, khm, kw] -> [(khm,kw), l, c, f]
    wh = weight.astype(np.float16).reshape(NF, L, KC, 4, KW)
    wt = np.ascontiguousarray(wh.transpose(3, 4, 1, 2, 0)).reshape(KP, L, KC, NF)
    wkf = np.zeros((KP, LPAD, KC, NF), np.float16)
    wkf[:, :L] = wt

    bkf = np.zeros((1, LPAD, NF), np.float16)
    bkf[0, :L] = bias.astype(np.float16).reshape(NF, L).T

    in_maps = []
    for c in range(NCORES):
        base = c * LPC
        m = {"bk": np.ascontiguousarray(bkf[:, base:base + LPC])}
        off = 0
        for p in range(NPAIR):
            s0, s1 = base + off, base + off + NPL[p]
            m[f"xp{p}"] = np.ascontiguousarray(xkf[:, s0:s1])
            m[f"wp{p}"] = np.ascontiguousarray(wkf[:, s0:s1])
            off += NPL[p]
        in_maps.append(m)
    return in_maps


def run(x, weight, bias, **run_kwargs):
    """Build+run; returns (output, BassKernelResults)."""
    if "nc" not in _CACHED:
        _CACHED["nc"] = _build_bass()
    nc = _CACHED["nc"]
    in_maps = _pack_inputs(x, weight, bias)
    res = bass_utils.run_bass_kernel_spmd(
        nc, in_maps, core_ids=list(range(NCORES)), **run_kwargs)
    # per core: op{p} is [NF, npl, B]; concat -> [NF, LPC, B]
    outs = np.stack([
        np.concatenate([r[f"op{p}"] for p in range(NPAIR)], axis=1)
        for r in res.results])                        # [8, NF, LPC, B]
    outs = outs.transpose(0, 2, 1, 3).reshape(LPAD, NF, B)[:L]  # [l, f, b]
    out = np.ascontiguousarray(outs.transpose(2, 1, 0)).reshape(B, NF, HS, WS)
    return out.astype(np.float32), res


def kernel(x, weight, bias):
    out, _ = run(x, weight, bias)
    return out


# revision 5
# speedup vs baseline: 3.0718x; 1.0400x over previous
"""Locally-connected Conv2d (unique weights per output location) on 8 trn2 cores.

Problem (hardcoded): x [256,1,280,280] f32, weight [12800,1,28,28] f32,
bias [12800,1] f32 -> out [256,128,10,10] f32.  kernel 28x28, stride 28
(non-overlapping patches), 10x10=100 locations, 128 filters.

Per location l the computation is a plain matmul:
    out[b, f, l] = sum_k patch[b, l, k] * w[f, l, k] + bias[f, l],  k in [0,784)

Strategy: shard the 100 locations across 8 cores (pad to 104 = 8*13).
Host-side we repack x into k-major patch layout and weights into k-major
filter layout (both fp16 to halve HBM traffic; accumulation is fp32 in
PSUM), so the device does nothing but streaming matmuls:
    per location: 7 accumulating matmuls [112k x 128f]^T @ [112k x 256b]
    + one K=1 matmul (ones x bias row) that folds in the bias.

The kernel is DMA-bound (~8.7 MB/core at ~270 GB/s effective).  Loads are
split per location-pair (7 pairs of ~1.2 MB) and drained in issue order
on the SWDGE queue, so compute on pair p overlaps the loads of pairs
p+1..; the single-location pair goes last to shorten the post-DMA tail.
Stores ride the HWDGE queues so each store's only sync-wait is its data
dependency; the last store goes SWDGE with its wait laundered through a
Pool carrier op.

Environment notes (this walrus build / axon runtime):
  - each DMA / matmul / ldweights / Pool-copy instruction may carry at
    most ONE sync-wait command; HW-DGE/SW-DGE lane groups have 8 lanes
    each and DMAs past the 8th on a group get a lane-reuse wait.
  - DMA partition counts MUST split evenly over the 16 SDMA engines
    (112 or 128): a 113-partition transfer shreds the leftover row into
    4-8 byte descriptors and collapses DMA bandwidth ~5x.
  - SWDGE (nc.gpsimd) sprays a dma_start across all 16 SDMA engines and
    queue packets drain in issue order; HWDGE handles 128-partition
    tiles fine but is not used for bulk loads here.
"""

import numpy as np

import concourse.bass as bass
import concourse.mybir as mybir
from concourse import bass_utils
from concourse.tile import TileContext
from concourse.vector_clock import ScopedClock


def _split_drain_and_barrier(self, tick_clock, wait_clock):
    """TileContext._drain_and_barrier with the tail drain's sem waits split
    across several drain instructions: this walrus build caps the number of
    sync-wait commands a single instruction may carry."""
    drain_inst = self.nc.sync.drain()
    wait_clock.add_sem_waits(
        drain_inst.ins, ScopedClock({None: tick_clock.global_clock}))
    mi = drain_inst.ins
    if mi.sync_info is not None and mi.sync_info.on_wait:
        waits = list(mi.sync_info.on_wait)
        ups = list(mi.sync_info.on_update or [])
        mi.sync_info = mybir.SyncInfo(on_wait=waits[:1], on_update=ups)
        for w in waits[1:]:
            extra = self.nc.sync.drain()
            extra.ins.sync_info = mybir.SyncInfo(on_wait=[w], on_update=[])
    self.nc.all_engine_barrier(sem_only=True)
    assert self.sems is not None
    popped = self.nc._tile_sem_poison_stack.pop()
    assert popped is self._sem_poison
    if not SKIP_TAIL_CLEAR:
        self.nc.clear_and_free_semaphores(list(self.sems.allocated().values()))
        self.nc.all_engine_barrier(sem_only=True)


SKIP_TAIL_CLEAR = True

TileContext._drain_and_barrier = _split_drain_and_barrier

B = 256       # batch
NF = 128      # filters
HS = WS = 10  # output spatial
L = HS * WS   # locations
KH = KW = 28  # kernel == stride (non-overlapping)
NCORES = 8
LPC = 13      # locations per core (8*13 = 104 >= 100, tail zero-padded)
LPAD = NCORES * LPC
KC = 7        # contraction chunks
KP = 112      # partitions per chunk (7*112 = 784); kh splits as (7,4)

# location pairs per core: 6 pairs of 2 + 1 single (pairs share a PSUM bank;
# the single-location pair is LAST so the post-DMA compute tail is short)
NPL = [2, 2, 2, 2, 2, 2, 1]
NPAIR = len(NPL)

_CACHED = {}


def _build_bass():
    nc = bass.Bass(trn_type="TRN2")
    xs = [nc.dram_tensor(f"xp{p}", [KP, NPL[p], KC, B], mybir.dt.float16,
                         kind="ExternalInput")
          for p in range(NPAIR)]
    ws = [nc.dram_tensor(f"wp{p}", [KP, NPL[p], KC, NF], mybir.dt.float16,
                         kind="ExternalInput")
          for p in range(NPAIR)]
    bk = nc.dram_tensor("bk", [1, LPC, NF], mybir.dt.float16,
                        kind="ExternalInput")
    # separate store tensors: avoids per-tensor WAW chaining between stores
    outs = [nc.dram_tensor(f"op{p}", [NF, NPL[p], B], mybir.dt.float16,
                           kind="ExternalOutput")
            for p in range(NPAIR)]

    with TileContext(nc) as tc:
        with (
            tc.tile_pool(name="xp", bufs=1) as xpool,
            tc.tile_pool(name="wp", bufs=1) as wpool,
            tc.tile_pool(name="bp", bufs=1) as bpool,
            tc.tile_pool(name="op", bufs=1) as opool,
            # 2 locations share one PSUM bank: NPAIR=7 tiles <= 8 banks, so
            # banks are never reused and matmuls need no release wait.
            tc.tile_pool(name="ps", bufs=1, space="PSUM") as pspool,
        ):
            ones_t = bpool.tile([1, B], mybir.dt.float16, tag="ones",
                                name="ones")
            nc.vector.memset(ones_t[:], 1.0)
            bias_t = bpool.tile([1, LPC, NF], mybir.dt.float16, tag="bias",
                                name="bias")
            # tiny; first in the SWDGE queue so it lands well before use
            nc.gpsimd.dma_start(bias_t[:], bk[:])

            # pair 0 rides the HWDGE queues: no Q7 descriptor-generation
            # spin-up, so its bytes start ~5 us before the SWDGE stream
            x_ts, w_ts = [], []
            for p in range(NPAIR):
                x_t = xpool.tile([KP, NPL[p], KC, B], mybir.dt.float16,
                                 tag=f"x{p}", name=f"x{p}")
                w_t = wpool.tile([KP, NPL[p], KC, NF], mybir.dt.float16,
                                 tag=f"w{p}", name=f"w{p}")
                if p == 0:
                    nc.sync.dma_start(x_t[:], xs[p][:])
                    nc.scalar.dma_start(w_t[:], ws[p][:])
                else:
                    nc.gpsimd.dma_start(x_t[:], xs[p][:])
                    nc.gpsimd.dma_start(w_t[:], ws[p][:])
                x_ts.append(x_t)
                w_ts.append(w_t)

            # HW-DGE lane budget is 8: 2 pair-0 loads + 6 stores.  The last
            # store goes SWDGE, its data wait laundered through a Pool
            # carrier op so the DMA carries only its lane-reuse wait.
            carrier = opool.tile([1, 1], mybir.dt.float16, tag="carrier",
                                 name="carrier")
            loc = 0
            for p in range(NPAIR):
                npl = NPL[p]
                ps = pspool.tile([NF, npl, B], mybir.dt.float32,
                                 tag=f"ps{p}", name=f"ps{p}")
                for j in range(npl):
                    for c in range(KC):
                        nc.tensor.matmul(ps[:, j, :], w_ts[p][:, j, c, :],
                                         x_ts[p][:, j, c, :],
                                         start=(c == 0), stop=False)
                    # bias: rank-1 update  ps[f, b] += bias[f] * 1
                    nc.tensor.matmul(ps[:, j, :], bias_t[:, loc + j, :],
                                     ones_t[:], start=False, stop=True)
                o_t = opool.tile([NF, npl, B], mybir.dt.float16,
                                 tag=f"o{p}", name=f"o{p}")
                nc.vector.tensor_copy(o_t[:], ps[:])
                if p < NPAIR - 1:
                    eng = nc.sync if p % 2 == 0 else nc.scalar
                    eng.dma_start(outs[p][:], o_t[:])
                else:
                    nc.gpsimd.tensor_copy(carrier[:], o_t[0:1, 0, 0:1])
                    nc.gpsimd.dma_start(outs[p][:], o_t[:])
                loc += npl
    return nc


def _pack_inputs(x, weight, bias):
    # x: [B,1,280,280] f32.  rows = i*28 + kh, kh = c*4 + khm; cols = j*28 + kw
    # xk[p, l=(i,j), c, b] fp16 with p = khm*28 + kw
    xh = x.astype(np.float16).reshape(B, HS, KC, 4, WS, KW)
    # (b, i, c, khm, j, kw) -> (khm, kw, i, j, c, b)
    xt = np.ascontiguousarray(xh.transpose(3, 5, 1, 4, 2, 0))
    xkf = np.zeros((KP, LPAD, KC, B), np.float16)
    xkf[:, :L] = xt.reshape(KP, L, KC, B)

    # weight: [NF*L, 1, 28, 28] -> [f, l, c, khm, kw] -> [(khm,kw), l, c, f]
    wh = weight.astype(np.float16).reshape(NF, L, KC, 4, KW)
    wt = np.ascontiguousarray(wh.transpose(3, 4, 1, 2, 0)).reshape(KP, L, KC, NF)
    wkf = np.zeros((KP, LPAD, KC, NF), np.float16)
    wkf[:, :L] = wt

    bkf = np.zeros((1, LPAD, NF), np.float16)
    bkf[0, :L] = bias.astype(np.float16).reshape(NF, L).T

    in_maps = []
    for c in range(NCORES):
        base = c * LPC
        m = {"bk": np.ascontiguousarray(bkf[:, base:base + LPC])}
        off = 0
        for p in range(NPAIR):
            s0, s1 = base + off, base + off + NPL[p]
            m[f"xp{p}"] = np.ascontiguousarray(xkf[:, s0:s1])
            m[f"wp{p}"] = np.ascontiguousarray(wkf[:, s0:s1])
            off += NPL[p]
        in_maps.append(m)
    return in_maps


def run(x, weight, bias, **run_kwargs):
    """Build+run; returns (output, BassKernelResults)."""
    if "nc" not in _CACHED:
        _CACHED["nc"] = _build_bass()
    nc = _CACHED["nc"]
    in_maps = _pack_inputs(x, weight, bias)
    res = bass_utils.run_bass_kernel_spmd(
        nc, in_maps, core_ids=list(range(NCORES)), **run_kwargs)
    # per core: op{p} is [NF, npl, B]; concat -> [NF, LPC, B]
    outs = np.stack([
        np.concatenate([r[f"op{p}"] for p in range(NPAIR)], axis=1)
        for r in res.results])                        # [8, NF, LPC, B]
    outs = outs.transpose(0, 2, 1, 3).reshape(LPAD, NF, B)[:L]  # [l, f, b]
    out = np.ascontiguousarray(outs.transpose(2, 1, 0)).reshape(B, NF, HS, WS)
    return out.astype(np.float32), res


def kernel(x, weight, bias):
    out, _ = run(x, weight, bias)
    return out
